# revision 63
# baseline (speedup 1.0000x reference)
"""AttentionGatedMamba on 8 trn2 NeuronCores (Bass/Tile, SPMD).

Sharding: 2 groups of 4 cores; group g owns batch b=g, rank r owns d_inner
channels [256r, 256r+256). Per core:
  p1  in_proj (bf16 MMs) + causal conv as 4 diagonal matmuls on TensorE
      + SiLU -> xc/z bf16; x_proj partials -> one bf16 AllReduce, with the
      z-block projection emitted after it so those MMs/silus run under the
      collective's latency.
  p2  dt = softplus(v) = ln(1+exp(v)) with the dt bias folded into the
      dt_proj matmul via a ones-row augmentation. u = dt*xc.
  p3  monolithic selective scan, all elementwise ops [128,2048] (measured
      DVE floor: TT bf16 1.22us, scan 4.4us regardless of dtype; chunking
      to 512/1024 or fusing to 4096 is slower). Per (state, e-block):
      dA = Exp(A_s*dt) f32 on the scalar engine, dBx = u*B bf16 on DVE,
      hardware scan, w = C*h bf16 on DVE; y = D*xc + sum_s w_s accumulates
      in f32 PSUM via identity/D-diagonal matmuls on the otherwise idle
      TensorE (GpSimd offload is counterproductive: its SBUF traffic
      stalls concurrent DVE ops 2-4x via shared ports). B/C rows are
      DMA-broadcast to 128 partitions on the sync HWDGE queue
      (dynamic-DGE queues take ~25us per broadcast; sync takes ~1us).
  p4  yb = y*silu_gate, out_proj partials (bf16 MMs) -> ReduceScatter
  p5  context gate on this core's 512-token slice.
"""
import numpy as np

import concourse.bass as bass  # noqa: F401
import concourse.mybir as mybir
from concourse import bacc, tile
from concourse.bass_utils import run_bass_kernel_spmd

F32 = mybir.dt.float32
BF16 = mybir.dt.bfloat16
AF = mybir.ActivationFunctionType
OP = mybir.AluOpType

B, L, D_MODEL = 2, 2048, 512
D_STATE, D_CONV = 16, 4
D_INNER = 2 * D_MODEL            # 1024
DT_RANK = 32
N_CORES = 8
GROUPS = [[0, 1, 2, 3], [4, 5, 6, 7]]
E_LOC = D_INNER // 4             # 256 channels per core
NEB = E_LOC // 128               # 2 e-blocks of 128
LS = L // 4                      # 512 output tokens per core

_CACHE = {}


def _build():
    nc = bacc.Bacc("TRN2", target_bir_lowering=False, debug=False,
                   enable_asserts=False, num_devices=N_CORES,
                   name="agmamba_v4")

    d_xT = nc.dram_tensor("xT", [D_MODEL, L], BF16, kind="ExternalInput")
    d_w1T = nc.dram_tensor("w1T", [D_MODEL, 2 * E_LOC], BF16, kind="ExternalInput")
    d_cwd = nc.dram_tensor("cwdiag", [(NEB * D_CONV + 3) * 128, 128], BF16,
                           kind="ExternalInput")
    d_cb = nc.dram_tensor("cb", [E_LOC, 1], F32, kind="ExternalInput")
    d_xpT = nc.dram_tensor("xpT", [E_LOC, 64], BF16, kind="ExternalInput")
    d_dtwT = nc.dram_tensor("dtwT", [DT_RANK + 1, E_LOC], BF16,
                            kind="ExternalInput")   # row 32 = dt bias
    d_A = nc.dram_tensor("Aneg", [E_LOC, D_STATE], F32, kind="ExternalInput")
    d_D = nc.dram_tensor("Dvec", [E_LOC, 1], F32, kind="ExternalInput")
    d_opT = nc.dram_tensor("opT", [E_LOC, D_MODEL], BF16, kind="ExternalInput")
    d_gwT = nc.dram_tensor("gwT", [2 * D_MODEL, D_MODEL], BF16, kind="ExternalInput")
    d_gb = nc.dram_tensor("gb", [D_MODEL, 1], F32, kind="ExternalInput")
    d_ctxT = nc.dram_tensor("ctxT", [D_MODEL, LS], BF16, kind="ExternalInput")
    d_out = nc.dram_tensor("out", [D_MODEL, LS], F32, kind="ExternalOutput")

    d_dtp_part = nc.dram_tensor("dtp_part", [DT_RANK, L], BF16)
    d_dtp_ar = nc.dram_tensor("dtp_ar", [DT_RANK, L], BF16)
    d_bcp_part = nc.dram_tensor("bcp_part", [2 * D_STATE, L], BF16)
    d_bcp_ar = nc.dram_tensor("bcp_ar", [2 * D_STATE, L], BF16)
    d_mp = nc.dram_tensor("m_part", [L, D_MODEL], BF16)
    d_mrs = nc.dram_tensor("m_rs", [LS, D_MODEL], BF16)

    with tile.TileContext(nc) as tc:
        with (
            tc.tile_pool(name="const", bufs=1) as cp,
            tc.tile_pool(name="persist", bufs=1) as pp,
        ):
            # ---- p1 input loads first (everything else can wait) ----
            xT_sb = []
            for k in range(4):
                t = cp.tile([128, L], BF16, tag=f"xT{k}", name=f"xT{k}")
                nc.sync.dma_start(t[:, :], d_xT[128 * k:128 * (k + 1), :])
                xT_sb.append(t)
            w1_sb = []
            for k in range(4):
                t = cp.tile([128, 2 * E_LOC], BF16, tag=f"w1{k}", name=f"w1{k}")
                nc.sync.dma_start(t[:, :], d_w1T[128 * k:128 * (k + 1), :])
                w1_sb.append(t)
            cwd_sb = [[None] * D_CONV for _ in range(NEB)]
            for eb in range(NEB):
                for tap in range(D_CONV):
                    i = eb * D_CONV + tap
                    t = cp.tile([128, 128], BF16, tag=f"cwd{i}", name=f"cwd{i}")
                    nc.sync.dma_start(t[:, :], d_cwd[128 * i:128 * (i + 1), :])
                    cwd_sb[eb][tap] = t
            ident_sb = cp.tile([128, 128], BF16, tag="ident", name="ident")
            nc.sync.dma_start(ident_sb[:, :],
                              d_cwd[128 * 8:128 * 9, :])
            Dd_sb = []
            for eb in range(NEB):
                t = cp.tile([128, 128], BF16, tag=f"Dd{eb}", name=f"Dd{eb}")
                nc.sync.dma_start(t[:, :], d_cwd[128 * (9 + eb):128 * (10 + eb), :])
                Dd_sb.append(t)
            xp_sb, dtw_sb = [], []
            cb_sb, A_sb, D_sb = [], [], []
            for eb in range(NEB):
                sl = slice(128 * eb, 128 * (eb + 1))
                t = cp.tile([128, 64], BF16, tag=f"xp{eb}", name=f"xps{eb}")
                nc.sync.dma_start(t[:, :], d_xpT[sl, :])
                xp_sb.append(t)
                t = cp.tile([DT_RANK + 1, 128], BF16, tag=f"dtw{eb}",
                            name=f"dtw{eb}")
                nc.sync.dma_start(t[:, :], d_dtwT[:, sl])
                dtw_sb.append(t)
                t = cp.tile([128, D_STATE], F32, tag=f"A{eb}", name=f"A{eb}")
                nc.sync.dma_start(t[:, :], d_A[sl, :])
                A_sb.append(t)
                for dst, src, tg in ((cb_sb, d_cb, "cb"), (D_sb, d_D, "D")):
                    t = cp.tile([128, 1], F32, tag=f"{tg}{eb}", name=f"{tg}{eb}")
                    nc.sync.dma_start(t[:, :], src[sl, :])
                    dst.append(t)
            op_sb = []
            for eb in range(NEB):
                t = cp.tile([128, D_MODEL], BF16, tag=f"op{eb}", name=f"op{eb}")
                nc.sync.dma_start(t[:, :], d_opT[128 * eb:128 * (eb + 1), :])
                op_sb.append(t)
            gwm_sb, gwc_sb, ctx_sb = [], [], []
            for k in range(4):
                t = cp.tile([128, D_MODEL], BF16, tag=f"gwm{k}", name=f"gwm{k}")
                nc.sync.dma_start(t[:, :], d_gwT[128 * k:128 * (k + 1), :])
                gwm_sb.append(t)
                t = cp.tile([128, D_MODEL], BF16, tag=f"gwc{k}", name=f"gwc{k}")
                nc.sync.dma_start(
                    t[:, :], d_gwT[D_MODEL + 128 * k:D_MODEL + 128 * (k + 1), :])
                gwc_sb.append(t)
                t = cp.tile([128, LS], BF16, tag=f"ctx{k}", name=f"ctx{k}")
                nc.sync.dma_start(t[:, :], d_ctxT[128 * k:128 * (k + 1), :])
                ctx_sb.append(t)
            gb_sb = cp.tile([128, 4], F32, tag="gb", name="gb_t")
            nc.sync.dma_start(gb_sb[:, :],
                              d_gb.ap().rearrange("(b a) c -> a (b c)", b=4))

            # ---- persistent activations (per e-block) ----
            xc = [pp.tile([128, L], BF16, tag=f"xc{eb}", name=f"xc{eb}")
                  for eb in range(NEB)]
            z_s = [pp.tile([128, L], BF16, tag=f"zs{eb}", name=f"zs{eb}")
                   for eb in range(NEB)]
            dt_sb = [pp.tile([128, L], BF16, tag=f"dt{eb}", name=f"dtt{eb}")
                     for eb in range(NEB)]
            u_sb = [pp.tile([128, L], BF16, tag=f"u{eb}", name=f"u{eb}")
                    for eb in range(NEB)]

            # ---- p1 ----
            with (
                nc.named_scope("p1_inproj"),
                tc.tile_pool(name="p1", bufs=1) as p1,
                tc.tile_pool(name="ps_inp", bufs=1, space="PSUM") as ps_inp,
                tc.tile_pool(name="ps_cnv", bufs=2, space="PSUM") as ps_cnv,
                tc.tile_pool(name="ps_xp", bufs=2, space="PSUM") as ps_xp,
            ):
                xi16 = [p1.tile([128, L + 3], BF16, tag=f"xi{eb}", name=f"xi{eb}")
                        for eb in range(NEB)]
                for eb in range(NEB):
                    nc.vector.memset(xi16[eb][:, 0:3], 0.0)

                def in_proj_mo(mo):
                    pss = [ps_inp.tile([128, 512], F32, tag=f"inp{t_}",
                                       name=f"inp{t_}") for t_ in range(4)]
                    for k in range(4):
                        for t_ in range(4):
                            csl = slice(512 * t_, 512 * (t_ + 1))
                            nc.tensor.matmul(
                                pss[t_][:, :],
                                w1_sb[k][:, 128 * mo:128 * (mo + 1)],
                                xT_sb[k][:, csl],
                                start=(k == 0), stop=(k == 3))
                    for t_ in range(4):
                        csl = slice(512 * t_, 512 * (t_ + 1))
                        if mo < 2:
                            nc.vector.tensor_scalar_mul(
                                xi16[mo][:, 3 + 512 * t_:3 + 512 * (t_ + 1)],
                                pss[t_][:, :], 1.0)
                        else:
                            nc.scalar.activation(z_s[mo - 2][:, csl],
                                                 pss[t_][:, :], AF.Silu)

                for mo in range(2):
                    in_proj_mo(mo)

                for eb in range(NEB):
                    for t_ in range(4):
                        csl = slice(512 * t_, 512 * (t_ + 1))
                        psc = ps_cnv.tile([128, 512], F32, tag="cnv",
                                          name=f"cnv{eb}_{t_}")
                        for tap in range(D_CONV):
                            nc.tensor.matmul(
                                psc[:, :], cwd_sb[eb][tap],
                                xi16[eb][:, tap + 512 * t_:tap + 512 * (t_ + 1)],
                                start=(tap == 0), stop=(tap == 3))
                        nc.scalar.activation(xc[eb][:, csl], psc[:, :],
                                             AF.Silu, bias=cb_sb[eb][:, 0:1])

                dtp_sb = p1.tile([DT_RANK, L], BF16, tag="dtp", name="dtp_t")
                bcp_sb = p1.tile([2 * D_STATE, L], BF16, tag="bcp", name="bcp_t")
                for t_ in range(4):
                    csl = slice(512 * t_, 512 * (t_ + 1))
                    ps = ps_xp.tile([64, 512], F32, tag="xp", name="xp_t")
                    for eb in range(NEB):
                        nc.tensor.matmul(ps[:, :], xp_sb[eb][:, :],
                                         xc[eb][:, csl],
                                         start=(eb == 0), stop=(eb == 1))
                    nc.vector.tensor_scalar_mul(dtp_sb[:, csl],
                                                ps[0:DT_RANK, :], 1.0)
                    nc.vector.tensor_scalar_mul(bcp_sb[:, csl],
                                                ps[DT_RANK:64, :], 1.0)
                nc.sync.dma_start(d_dtp_part[:, :], dtp_sb[:, :])
                nc.sync.dma_start(d_bcp_part[:, :], bcp_sb[:, :])

                # z blocks emitted before the collective so their MMs/silus
                # can run under the AllReduce
                for mo in range(2, 4):
                    in_proj_mo(mo)

                with nc.named_scope("ar_dt"):
                    nc.gpsimd.collective_compute(
                        "AllReduce", OP.add, replica_groups=GROUPS,
                        ins=[d_dtp_part.ap().opt()],
                        outs=[d_dtp_ar.ap().opt()])
                with nc.named_scope("ar_bc"):
                    nc.gpsimd.collective_compute(
                        "AllReduce", OP.add, replica_groups=GROUPS,
                        ins=[d_bcp_part.ap().opt()],
                        outs=[d_bcp_ar.ap().opt()])

            # ---- p2: dt = ln(1+exp(v+b)), bias via ones-row in the MM ----
            with (
                nc.named_scope("p2_dt"),
                tc.tile_pool(name="p2", bufs=1) as p2,
                tc.tile_pool(name="ps_dt", bufs=4, space="PSUM") as ps_dt,
            ):
                dtlow = p2.tile([DT_RANK + 1, L], BF16, tag="dtlow",
                                name="dtlow_t")
                nc.sync.dma_start(dtlow[0:DT_RANK, :], d_dtp_ar[:, :])
                nc.vector.memset(dtlow[DT_RANK:DT_RANK + 1, :], 1.0)
                for eb in range(NEB):
                    vsb = p2.tile([128, L], F32, tag=f"vsb{eb}",
                                  name=f"vsb{eb}")
                    ev = p2.tile([128, L], F32, tag=f"ev{eb}", name=f"ev{eb}")
                    for t_ in range(4):
                        ps = ps_dt.tile([128, 512], F32, tag="dtp",
                                        name=f"dtp{eb}_{t_}")
                        nc.tensor.matmul(ps[:, :], dtw_sb[eb][:, :],
                                         dtlow[:, 512 * t_:512 * (t_ + 1)],
                                         start=True, stop=True)
                        nc.vector.tensor_scalar_mul(
                            vsb[:, 512 * t_:512 * (t_ + 1)], ps[:, :], 1.0)
                    ev2 = ev
                    nc.scalar.activation(ev2[:, :], vsb[:, :], AF.Exp)
                    nc.scalar.activation(dt_sb[eb][:, :], ev2[:, :], AF.Ln,
                                         bias=1.0)
                    nc.vector.tensor_tensor(u_sb[eb][:, :], dt_sb[eb][:, :],
                                            xc[eb][:, :], OP.mult)

            # ---- p3: monolithic scan, all [128,2048] ops ----
            with (
                nc.named_scope("p3_scan"),
                tc.tile_pool(name="bcb", bufs=2) as bcb,
                tc.tile_pool(name="sw", bufs=5) as sw,
                tc.tile_pool(name="swa", bufs=3) as swa,
                tc.tile_pool(name="sw2", bufs=3) as sw2,
                tc.tile_pool(name="yacc", bufs=1) as yap,
                tc.tile_pool(name="ps_y", bufs=1, space="PSUM") as ps_y,
            ):
                # y accumulated in PSUM by TensorE via diagonal matmuls
                yps = [ps_y.tile([128, L], F32, tag=f"y{eb}", name=f"yps{eb}")
                       for eb in range(NEB)]
                for eb in range(NEB):
                    for t_ in range(4):
                        csl = slice(512 * t_, 512 * (t_ + 1))
                        nc.tensor.matmul(yps[eb][:, csl], Dd_sb[eb],
                                         xc[eb][:, csl], start=True, stop=False)

                for sp in range(D_STATE // 2):
                    s0, s1 = 2 * sp, 2 * sp + 1
                    bbs, cbs = {}, {}
                    for s in (s0, s1):
                        bb = bcb.tile([128, L], BF16, tag="bb", name=f"bb{s}")
                        nc.sync.dma_start(
                            bb[:, :],
                            d_bcp_ar[s:s + 1, :].to_broadcast([128, L]))
                        cbr = bcb.tile([128, L], BF16, tag="cb", name=f"cb{s}")
                        nc.sync.dma_start(
                            cbr[:, :],
                            d_bcp_ar[D_STATE + s:D_STATE + s + 1, :]
                            .to_broadcast([128, L]))
                        bbs[s], cbs[s] = bb, cbr
                    dAs, dBxs, hs = {}, {}, {}
                    for s in (s0, s1):
                        for eb in range(NEB):
                            dA = swa.tile([128, L], F32, tag="dA",
                                          name=f"dA{s}_{eb}")
                            nc.scalar.activation(dA[:, :], dt_sb[eb][:, :],
                                                 AF.Exp,
                                                 scale=A_sb[eb][:, s:s + 1])
                            dAs[(s, eb)] = dA
                    for s in (s0, s1):
                        for eb in range(NEB):
                            dBx = sw.tile([128, L], BF16, tag="dBx",
                                          name=f"dBx{s}_{eb}")
                            nc.vector.tensor_tensor(dBx[:, :], u_sb[eb][:, :],
                                                    bbs[s][:, :], OP.mult)
                            dBxs[(s, eb)] = dBx
                    for s in (s0, s1):
                        for eb in range(NEB):
                            h = sw.tile([128, L], BF16, tag="h",
                                        name=f"h{s}_{eb}")
                            nc.vector.tensor_tensor_scan(
                                h[:, :], dAs[(s, eb)][:, :],
                                dBxs[(s, eb)][:, :], 0.0, OP.mult, OP.add)
                            hs[(s, eb)] = h
                    for s in (s0, s1):
                        for eb in range(NEB):
                            w = sw2.tile([128, L], BF16, tag=f"w{eb}",
                                         name=f"w{s}_{eb}")
                            nc.vector.tensor_tensor(
                                w[:, :], hs[(s, eb)][:, :],
                                cbs[s][:, :], OP.mult)
                            last = (s == D_STATE - 1)
                            for t_ in range(4):
                                csl = slice(512 * t_, 512 * (t_ + 1))
                                nc.tensor.matmul(yps[eb][:, csl], ident_sb,
                                                 w[:, csl],
                                                 start=False, stop=last)

                yb = []
                for eb in range(NEB):
                    t = yap.tile([128, L], BF16, tag=f"yb{eb}", name=f"yb{eb}")
                    nc.vector.tensor_tensor(t[:, :], yps[eb][:, :],
                                            z_s[eb][:, :], OP.mult)
                    yb.append(t)

            # ---- p4: out_proj partial + ReduceScatter ----
            with (
                nc.named_scope("p4_op"),
                tc.tile_pool(name="p4w", bufs=3) as p4w,
                tc.tile_pool(name="ps_op", bufs=2, space="PSUM") as ps_op,
            ):
                for tt in range(L // 128):
                    ps = ps_op.tile([128, D_MODEL], F32, tag="op",
                                    name=f"op{tt}")
                    for eb in range(NEB):
                        nc.tensor.matmul(
                            ps[:, :], yb[eb][:, 128 * tt:128 * (tt + 1)],
                            op_sb[eb][:, :], start=(eb == 0), stop=(eb == 1))
                    msb = p4w.tile([128, D_MODEL], BF16, tag="msb",
                                   name=f"msb{tt}")
                    nc.scalar.activation(msb[:, :], ps[:, :], AF.Copy)
                    nc.sync.dma_start(d_mp[128 * tt:128 * (tt + 1), :],
                                      msb[:, :])

            with nc.named_scope("rs_m"):
                nc.gpsimd.collective_compute(
                    "ReduceScatter", OP.add, replica_groups=GROUPS,
                    ins=[d_mp.ap().opt()], outs=[d_mrs.ap().opt()])

            # ---- p5: gate ----
            with (
                nc.named_scope("p5_gate"),
                tc.tile_pool(name="p5w", bufs=2) as p5w,
                tc.tile_pool(name="p5m", bufs=1) as p5m,
                tc.tile_pool(name="ps_g5", bufs=2, space="PSUM") as ps_g5,
            ):
                mT = []
                for k in range(4):
                    t = p5m.tile([128, LS], BF16, tag=f"mT{k}", name=f"mT{k}")
                    nc.sync.dma_start_transpose(
                        t[:, :], d_mrs[:, 128 * k:128 * (k + 1)])
                    mT.append(t)
                for mo in range(4):
                    ps = ps_g5.tile([128, LS], F32, tag="g5", name=f"g5{mo}")
                    for k in range(4):
                        nc.tensor.matmul(ps[:, :],
                                         gwm_sb[k][:, 128 * mo:128 * (mo + 1)],
                                         mT[k][:, :],
                                         start=(k == 0), stop=False)
                    for k in range(4):
                        nc.tensor.matmul(ps[:, :],
                                         gwc_sb[k][:, 128 * mo:128 * (mo + 1)],
                                         ctx_sb[k][:, :],
                                         start=False, stop=(k == 3))
                    g = p5w.tile([128, LS], F32, tag="g", name=f"g{mo}")
                    nc.scalar.activation(g[:, :], ps[:, :], AF.Sigmoid,
                                         bias=gb_sb[:, mo:mo + 1])
                    o = p5w.tile([128, LS], F32, tag="o", name=f"o{mo}")
                    nc.vector.tensor_tensor(o[:, :], mT[mo][:, :], g[:, :],
                                            OP.mult)
                    nc.sync.dma_start(d_out[128 * mo:128 * (mo + 1), :],
                                      o[:, :])

    nc.compile()
    return nc


def _prep_in_maps(inputs):
    x = np.asarray(inputs["x"], np.float32)
    context = np.asarray(inputs["context"], np.float32)
    in_proj_w = np.asarray(inputs["in_proj_w"], np.float32)
    conv_w = np.asarray(inputs["conv_w"], np.float32)
    conv_b = np.asarray(inputs["conv_b"], np.float32)
    x_proj_w = np.asarray(inputs["x_proj_w"], np.float32)
    dt_proj_w = np.asarray(inputs["dt_proj_w"], np.float32)
    dt_proj_b = np.asarray(inputs["dt_proj_b"], np.float32)
    A_log = np.asarray(inputs["A_log"], np.float32)
    Dv = np.asarray(inputs["D"], np.float32)
    out_proj_w = np.asarray(inputs["out_proj_w"], np.float32)
    gate_w = np.asarray(inputs["gate_w"], np.float32)
    gate_b = np.asarray(inputs["gate_b"], np.float32)

    import ml_dtypes
    bf16 = ml_dtypes.bfloat16

    gwT = np.ascontiguousarray(gate_w.T).astype(bf16)      # [1024, 512]
    gb = np.ascontiguousarray(gate_b[:, None])             # [512, 1]
    Aneg_full = -np.exp(A_log)                             # true A (negative)

    in_maps = []
    for core in range(N_CORES):
        g, r = divmod(core, 4)
        er = slice(E_LOC * r, E_LOC * (r + 1))
        w1 = np.concatenate([in_proj_w[er],
                             in_proj_w[D_INNER + E_LOC * r:
                                       D_INNER + E_LOC * (r + 1)]], 0)
        cwd = np.zeros((NEB * D_CONV + 3, 128, 128), np.float32)
        cwl = conv_w[er]                                   # [256, 4]
        for eb in range(NEB):
            for tap in range(D_CONV):
                np.fill_diagonal(cwd[eb * D_CONV + tap],
                                 cwl[128 * eb:128 * (eb + 1), tap])
        np.fill_diagonal(cwd[8], 1.0)                      # identity
        for eb in range(NEB):                              # D diagonals
            np.fill_diagonal(cwd[9 + eb],
                             Dv[er][128 * eb:128 * (eb + 1)])
        dtw_aug = np.concatenate([dt_proj_w[er].T,
                                  dt_proj_b[er][None, :]], 0)  # [33, 256]
        m = {
            "xT": np.ascontiguousarray(x[g].T).astype(bf16),
            "w1T": np.ascontiguousarray(w1.T).astype(bf16),
            "cwdiag": np.ascontiguousarray(
                cwd.reshape((NEB * D_CONV + 3) * 128, 128)).astype(bf16),
            "cb": np.ascontiguousarray(conv_b[er][:, None]),
            "xpT": np.ascontiguousarray(x_proj_w[:, er].T).astype(bf16),
            "dtwT": np.ascontiguousarray(dtw_aug).astype(bf16),
            "Aneg": np.ascontiguousarray(Aneg_full[er]),
            "Dvec": np.ascontiguousarray(Dv[er][:, None]),
            "opT": np.ascontiguousarray(out_proj_w[:, er].T).astype(bf16),
            "gwT": gwT,
            "gb": gb,
            "ctxT": np.ascontiguousarray(
                context[g, LS * r:LS * (r + 1), :].T).astype(bf16),
        }
        in_maps.append(m)
    return in_maps


def _unshard(results):
    out = np.zeros((B, L, D_MODEL), np.float32)
    for core in range(N_CORES):
        g, r = divmod(core, 4)
        out[g, LS * r:LS * (r + 1), :] = results[core]["out"].T
    return out


def kernel(**inputs):
    if "nc" not in _CACHE:
        _CACHE["nc"] = _build()
    nc = _CACHE["nc"]
    in_maps = _prep_in_maps(inputs)
    res = run_bass_kernel_spmd(nc, in_maps, core_ids=list(range(N_CORES)))
    return _unshard(res.results)
